# revision 21
# baseline (speedup 1.0000x reference)
"""Trainium2 Bass kernel for nn_Decay2DBlk (block-decay linear attention).

Full-input contract: kernel(**inputs) takes the unsharded inputs from
setup_inputs() and returns the full [B, T, E] output.

Sharding: 8 cores = 4 batch elements x 2 Dv-halves. Each core computes a
partial output y_b_h = (attn(x_b) restricted to its Dv half) @ Wo[half];
the host sums the two partials per batch element (the "all-reduce after
w_out" done host-side since outputs are gathered anyway).

Math (per core): with d=0.99, D=d^128, block index n, in-block offsets
s,t (keys/queries):
  out[t] = sum_{s<=t} d^(t_g - s_g) * q_t k_s * v_s  @ Wo      (t_g global)
All decay factors are folded into host-precomputed constants so the device
only does matmuls + one elementwise mask multiply + a running-sum state:
  - masks[n][s,t]  = 1[s<=t] d^(-s-1) D^-(n-1)        (intra-block, bf16)
  - vscale[n][t]   = (1-d) d^(127-t) D^-n             (v' for state update)
  - escale[n][t]   = d^(t+1) D^(n-1)                  (final ACT evac scale)
  - state S~ = sum_m k_m^T v'_m  (pure running sum, bf16 in SBUF)
The geometric growth of D^-n keeps every intermediate within fp32/bf16
range (max ~1e19) and makes bf16 rounding of the running sum benign
(validated: rel err ~4e-3 vs fp32 reference).

Perf structure (v3):
  - supertiles 0/1 compute BOTH q/k dk-halves locally (skips the first
    AllGathers, which otherwise head-of-line block the PE for ~28us);
    supertiles 2..7 split q/k across the pair and exchange via AllGather
    overlapped with compute (kT gather on the sync ring, qT on gpsimd).
  - state S~ updated at block-PAIR granularity: the kv matmuls of blocks
    (2m, 2m+1) accumulate in PSUM, halving the DVE tensor-adds that
    otherwise rate-limit the PE through the PSUM-bank rotation. Odd
    blocks add the missing q_n kv_{n-1} term as vp_{n-1}^T (k_{n-1} q_n^T)
    (mask-free: the vscale/escale constants supply (1-d) d^(128+t-s)).
  - inputs spread across the three DMA-capable rings (sync/scalar/gpsimd,
    ~76 GB/s each) in need-order; out stores are bf16 (host upcasts).
  - out-projection software-pipelined one block behind the y matmuls.
"""

import os
import sys

for _p in (
    "/root/.axon_site",
    "/root/.axon_site/_ro/trn_rl_repo",
    "/root/.axon_site/_ro/pypackages",
    "/opt/trn_rl_repo",
):
    if os.path.isdir(_p) and _p not in sys.path:
        sys.path.append(_p)

import numpy as np
import ml_dtypes
from contextlib import ExitStack

import concourse.bacc as bacc
import concourse.tile as tile
from concourse import mybir
from concourse.bass_utils import run_bass_kernel_spmd

DECAY = 0.99
TBLK = 128
BF16 = ml_dtypes.bfloat16
E4M3 = ml_dtypes.float8_e4m3
BF = mybir.dt.bfloat16
F32 = mybir.dt.float32
F8 = mybir.dt.float8e4
# fp8 scale on Wq/Wk so the N(0, 0.02^2) weights land in e4m3's normal
# range (std 1.28); undone by the sigmoid activation's scale=1/WSCALE
WSCALE = 64.0


def build_nc(T=4096, E=1024, Dk=1024, Dvh=512, ST=512, pair_groups=None):
    """Build the per-core Bass program. Same program runs on all 8 cores
    (SPMD); only the input data differs.

    For u>=2, q/k phase-A work is split across the two cores of a pair:
    each core computes sigmoid(x @ Wq_half + b_half) for the dk-half whose
    weights it was GIVEN as input (wq input is [E, Dk/2]), then the halves
    are exchanged with a pairwise AllGather through DRAM bounce buffers.
    Core 2b (group rank 0) always carries the low half, so the gathered
    layout is identical on both cores and the program stays SPMD-symmetric.
    Supertiles 0 and 1 instead compute the full Dk locally from wqf/wkf
    (same global chunk order as the gathered layout) so no collective sits
    on the critical path at startup."""
    N = T // TBLK       # number of 128-token blocks
    NU = T // ST        # number of super-tiles
    JB = ST // TBLK     # blocks per super-tile
    EC = E // 128       # E chunks (contraction)
    KC = Dk // 128      # Dk chunks
    HC = KC // 2        # dk chunks computed locally (half)
    DC = Dvh // 128     # Dv-half chunks
    Dkh = Dk // 2
    if pair_groups is None:
        pair_groups = [[0, 1], [2, 3], [4, 5], [6, 7]]

    nc = bacc.Bacc(num_devices=8)
    # all inputs host-pre-arranged to [128-partition, ...contiguous] layout so
    # every DMA is 128 descriptors of 4-8KB (max descriptor efficiency)
    xt = nc.dram_tensor("xt", [NU, 128, EC, ST], BF, kind="ExternalInput")
    xt8 = nc.dram_tensor("xt8", [NU, 128, EC, ST], F8, kind="ExternalInput")
    # full q/k weights (global chunk order, identical on both pair cores)
    wqf = nc.dram_tensor("wqf", [2, 128, EC, Dkh], F8, kind="ExternalInput")
    wkf = nc.dram_tensor("wkf", [2, 128, EC, Dkh], F8, kind="ExternalInput")
    bqf = nc.dram_tensor("bqf", [128, KC], F32, kind="ExternalInput")
    bkf = nc.dram_tensor("bkf", [128, KC], F32, kind="ExternalInput")
    # per-core halves (this core's share for the u>=2 split phase A)
    wq = nc.dram_tensor("wq", [128, EC, Dkh], F8, kind="ExternalInput")
    wk = nc.dram_tensor("wk", [128, EC, Dkh], F8, kind="ExternalInput")
    wv = nc.dram_tensor("wv", [128, EC, Dvh], BF, kind="ExternalInput")
    wo = nc.dram_tensor("wo", [128, DC, E], BF, kind="ExternalInput")
    bq = nc.dram_tensor("bq", [128, HC], F32, kind="ExternalInput")
    bk = nc.dram_tensor("bk", [128, HC], F32, kind="ExternalInput")
    masks = nc.dram_tensor("masks", [128, N, 128], BF, kind="ExternalInput")
    vscale = nc.dram_tensor("vscale", [128, N], F32, kind="ExternalInput")
    escale = nc.dram_tensor("escale", [128, N], F32, kind="ExternalInput")
    ident = nc.dram_tensor("ident", [128, 128], BF, kind="ExternalInput")
    out = nc.dram_tensor("out", [T, E], BF, kind="ExternalOutput")

    SIG = mybir.ActivationFunctionType.Sigmoid
    CPY = mybir.ActivationFunctionType.Copy

    with tile.TileContext(nc) as tc:
        with ExitStack() as ctx:
            consts = ctx.enter_context(tc.tile_pool(name="consts", bufs=1))
            qkh_pool = ctx.enter_context(tc.tile_pool(name="qkh", bufs=2))
            dram = ctx.enter_context(tc.tile_pool(name="dram", bufs=2, space="DRAM"))
            xt_pool = ctx.enter_context(tc.tile_pool(name="xt", bufs=3))
            qk_pool = ctx.enter_context(tc.tile_pool(name="qk", bufs=3))
            v_pool = ctx.enter_context(tc.tile_pool(name="v", bufs=3))
            kn_pool = ctx.enter_context(tc.tile_pool(name="kn", bufs=3))
            ap_pool = ctx.enter_context(tc.tile_pool(name="apool", bufs=3))
            yt_pool = ctx.enter_context(tc.tile_pool(name="yt", bufs=3))
            o_pool = ctx.enter_context(tc.tile_pool(name="opool", bufs=2))
            st_pool = ctx.enter_context(tc.tile_pool(name="state", bufs=1))
            psA = ctx.enter_context(tc.tile_pool(name="psA", bufs=2, space="PSUM"))
            psB = ctx.enter_context(tc.tile_pool(name="psB", bufs=3, space="PSUM"))
            psY = ctx.enter_context(tc.tile_pool(name="psY", bufs=2, space="PSUM"))
            psa = ctx.enter_context(tc.tile_pool(name="psa", bufs=1, space="PSUM"))

            # ---- constants into SBUF ----
            # One trigger-ring queue sustains only ~76 GB/s, so the inputs
            # are spread across the three DMA-capable rings in need-order:
            #   sync ring:   xt8[0], wqf halves, xt8[1], wq, wk,
            #                then per-supertile xt8[u] + gathered-kT loads
            #   scalar ring: biases, wkf halves, early masks, scales,
            #                then per-supertile xt[u] (bf16) + out stores
            #   gpsimd ring: ident, xt[0], wv, wo, then AllGather bounces
            # q/k full weights split by OUTPUT columns (dk-halves) AND
            # into e-pair tiles, DMA-interleaved with the matching xt8[0]
            # e-pairs: the very first q matmul is gated on ~0.26 MB.
            EH2 = EC // 2
            xt8_0c = [consts.tile([128, 2, ST], F8, name=f"xt80_{e}")
                      for e in range(EH2)]
            wqf_sb = [[consts.tile([128, 2, Dkh], F8, name=f"wqf{h}_{e}")
                       for e in range(EH2)] for h in range(2)]
            wkf_sb = [[consts.tile([128, 2, Dkh], F8, name=f"wkf{h}_{e}")
                       for e in range(EH2)] for h in range(2)]
            bqf_sb = consts.tile([128, KC], F32)
            nc.scalar.dma_start(out=bqf_sb, in_=bqf[:, :])
            bkf_sb = consts.tile([128, KC], F32)
            nc.scalar.dma_start(out=bkf_sb, in_=bkf[:, :])
            bq_sb = consts.tile([128, HC], F32)
            nc.scalar.dma_start(out=bq_sb, in_=bq[:, :])
            bk_sb = consts.tile([128, HC], F32)
            nc.scalar.dma_start(out=bk_sb, in_=bk[:, :])
            for e in range(EH2):
                nc.sync.dma_start(out=wqf_sb[0][e], in_=wqf[0, :, 2 * e:2 * e + 2, :])
                nc.sync.dma_start(out=xt8_0c[e], in_=xt8[0, :, 2 * e:2 * e + 2, :])
                nc.scalar.dma_start(out=wkf_sb[0][e], in_=wkf[0, :, 2 * e:2 * e + 2, :])
            for e in range(EH2):
                nc.sync.dma_start(out=wqf_sb[1][e], in_=wqf[1, :, 2 * e:2 * e + 2, :])
                nc.scalar.dma_start(out=wkf_sb[1][e], in_=wkf[1, :, 2 * e:2 * e + 2, :])
            xt8_1 = xt_pool.tile([128, EC, ST], F8, name="xt8_u", tag="xt8")
            nc.sync.dma_start(out=xt8_1, in_=xt8[1])
            # u>=2 half weights behind xt8[1] on the sync ring
            wq_sb = consts.tile([128, EC, Dkh], F8)
            nc.sync.dma_start(out=wq_sb, in_=wq[:, :, :])
            wk_sb = consts.tile([128, EC, Dkh], F8)
            nc.sync.dma_start(out=wk_sb, in_=wk[:, :, :])
            # masks split so the first two supertiles' worth arrives early;
            # the rest is triggered later (inside phase_a(3)) so it never
            # delays the per-supertile xt streams
            NM0 = 2 * JB
            mk0_sb = consts.tile([128, NM0, 128], BF)
            nc.scalar.dma_start(out=mk0_sb, in_=masks[:, :NM0, :])
            vs_sb = consts.tile([128, N], F32)
            nc.scalar.dma_start(out=vs_sb, in_=vscale[:, :])
            es_sb = consts.tile([128, N], F32)
            nc.scalar.dma_start(out=es_sb, in_=escale[:, :])
            mk1_sb = consts.tile([128, N - NM0, 128], BF)
            id_sb = consts.tile([128, 128], BF)
            nc.gpsimd.dma_start(out=id_sb, in_=ident[:, :])
            xt_0 = xt_pool.tile([128, EC, ST], BF, name="xt_u", tag="xt")
            nc.gpsimd.dma_start(out=xt_0, in_=xt[0])
            wv_sb = consts.tile([128, EC, Dvh], BF)
            nc.gpsimd.dma_start(out=wv_sb, in_=wv[:, :, :])
            wo_sb = consts.tile([128, DC, E], BF)
            nc.gpsimd.dma_start(out=wo_sb, in_=wo[:, :, :])

            def mk(n):
                return mk0_sb[:, n, :] if n < NM0 else mk1_sb[:, n - NM0, :]

            # persistent scaled-sum state S~ [dk, dv], one tile per dk-chunk
            S_c = [st_pool.tile([128, Dvh], BF, name=f"S{c}", tag=f"S{c}")
                   for c in range(KC)]

            def emit_phase_a_full(u, xt8_u):
                # ---- supertiles 0/1: full q/k computed locally, no
                # AllGather on the startup critical path.
                if u == 0:
                    xt_u = xt_0
                else:
                    xt_u = xt_pool.tile([128, EC, ST], BF, name="xt_u", tag="xt")
                    nc.scalar.dma_start(out=xt_u, in_=xt[u])
                qT_u = qk_pool.tile([128, KC, ST], BF, name="qT_u", tag="qT")
                kT_u = qk_pool.tile([128, KC, ST], BF, name="kT_u", tag="kT")
                for wsb, bsb, dst in ((wqf_sb, bqf_sb, qT_u), (wkf_sb, bkf_sb, kT_u)):
                    for c in range(KC):
                        csl = slice((c % (KC // 2)) * 128, (c % (KC // 2) + 1) * 128)
                        ps = psA.tile([128, ST], F32, name="psq", tag="psA")
                        for e in range(0, EC, 2):
                            nc.tensor.matmul(
                                ps, wsb[c // (KC // 2)][e // 2][:, :, csl],
                                xt8_u[e // 2] if u == 0 else xt8_u[:, e:e + 2, :],
                                start=(e == 0), stop=(e == EC - 2),
                                perf_mode=mybir.MatmulPerfMode.DoubleRow)
                        nc.scalar.activation(dst[:, c, :], ps, SIG,
                                             bias=bsb[:, c:c + 1], scale=1.0 / WSCALE)
                return qT_u, kT_u, xt_u

            def emit_v(u, tiles):
                # ---- v projection for supertile u (bf16: fp8 x would put
                # ~3.7% error straight onto the value path). Emitted right
                # before its block loop; the xt[u] bf16 stream was triggered
                # two supertiles earlier.
                qT_u, kT_u, xt_u = tiles
                v_u = v_pool.tile([128, JB, Dvh], BF, name="v_u", tag="v")
                vp_u = v_pool.tile([128, JB, Dvh], BF, name="vp_u", tag="vp")
                for j in range(JB):
                    n = u * JB + j
                    ps = psA.tile([128, ST], F32, name="psv", tag="psA")[:, :Dvh]
                    for e in range(EC):
                        nc.tensor.matmul(
                            ps, xt_u[:, e, j * 128:(j + 1) * 128], wv_sb[:, e, :],
                            start=(e == 0), stop=(e == EC - 1))
                    nc.scalar.activation(v_u[:, j, :], ps, CPY, scale=1.0 - DECAY)
                    nc.scalar.activation(vp_u[:, j, :], ps, CPY, scale=vs_sb[:, n:n + 1])
                return qT_u, kT_u, v_u, vp_u

            def emit_phase_a(u):
                # xt streams for supertile u ride the sync (fp8) and scalar
                # (bf16) rings, triggered here = two supertiles early.
                xt8_u = xt_pool.tile([128, EC, ST], F8, name="xt8_u", tag="xt8")
                nc.sync.dma_start(out=xt8_u, in_=xt8[u])
                xt_u = xt_pool.tile([128, EC, ST], BF, name="xt_u", tag="xt")
                nc.scalar.dma_start(out=xt_u, in_=xt[u])
                if u == 3:
                    nc.scalar.dma_start(out=mk1_sb, in_=masks[:, NM0:, :])

                # ---- phase A: this core computes its dk-half of qT, kT ----
                # fp8 DoubleRow: two 128-deep contraction subtiles per pass
                qTh_u = qkh_pool.tile([128, HC, ST], BF, name="qTh_u", tag="qTh")
                kTh_u = qkh_pool.tile([128, HC, ST], BF, name="kTh_u", tag="kTh")
                for c in range(HC):
                    csl = slice(c * 128, (c + 1) * 128)
                    ps = psA.tile([128, ST], F32, name="psq", tag="psA")
                    for e in range(0, EC, 2):
                        nc.tensor.matmul(
                            ps, wq_sb[:, e:e + 2, csl], xt8_u[:, e:e + 2, :],
                            start=(e == 0), stop=(e == EC - 2),
                            perf_mode=mybir.MatmulPerfMode.DoubleRow)
                    nc.scalar.activation(qTh_u[:, c, :], ps, SIG,
                                         bias=bq_sb[:, c:c + 1], scale=1.0 / WSCALE)
                for c in range(HC):
                    csl = slice(c * 128, (c + 1) * 128)
                    ps = psA.tile([128, ST], F32, name="psk", tag="psA")
                    for e in range(0, EC, 2):
                        nc.tensor.matmul(
                            ps, wk_sb[:, e:e + 2, csl], xt8_u[:, e:e + 2, :],
                            start=(e == 0), stop=(e == EC - 2),
                            perf_mode=mybir.MatmulPerfMode.DoubleRow)
                    nc.scalar.activation(kTh_u[:, c, :], ps, SIG,
                                         bias=bk_sb[:, c:c + 1], scale=1.0 / WSCALE)

                # ---- exchange halves with the pair partner (AllGather) ----
                # bounce-out + qT gather ride gpsimd; the kT gather rides the
                # sync ring (its AllGather-completion wait resolves ~a whole
                # supertile before anything queued behind it is needed)
                bin_u = dram.tile([2, 128, HC, ST], BF, name="bin_u", tag="bin")
                nc.gpsimd.dma_start(out=bin_u[0], in_=qTh_u)
                nc.gpsimd.dma_start(out=bin_u[1], in_=kTh_u)
                bout_u = dram.tile([2, 2, 128, HC, ST], BF, name="bout_u", tag="bout")
                nc.gpsimd.collective_compute(
                    "AllGather", mybir.AluOpType.bypass,
                    replica_groups=pair_groups,
                    ins=[bin_u.opt()], outs=[bout_u.opt()])
                qT_u = qk_pool.tile([128, KC, ST], BF, name="qT_u", tag="qT")
                kT_u = qk_pool.tile([128, KC, ST], BF, name="kT_u", tag="kT")
                for g in range(2):
                    nc.sync.dma_start(
                        out=kT_u[:, g * HC:(g + 1) * HC, :], in_=bout_u[g, 1])
                    nc.gpsimd.dma_start(
                        out=qT_u[:, g * HC:(g + 1) * HC, :], in_=bout_u[g, 0])
                return qT_u, kT_u, xt_u

            # out-projection is software-pipelined one block behind: the
            # yT evac (scalar) gets the next block a/kn matmuls to finish
            # under before the out matmuls need it as stationary.
            prev = [None, None]   # (yT_sb, n)

            def emit_out(prev_yT, pn):
                # out[t, e] = yT^T @ Wo, evacuated with escale[n][t]
                o_sb = o_pool.tile([128, E], BF, name="o_sb")
                for hh in range(E // 512):
                    o_ps = psY.tile([128, 4 * 128], F32, name="o_ps", tag="psY")
                    for dc in range(DC):
                        nc.tensor.matmul(
                            o_ps, prev_yT[:, dc * 128:(dc + 1) * 128],
                            wo_sb[:, dc, hh * 512:(hh + 1) * 512],
                            start=(dc == 0), stop=(dc == DC - 1))
                    nc.scalar.activation(
                        o_sb[:, hh * 512:(hh + 1) * 512], o_ps, CPY,
                        scale=es_sb[:, pn:pn + 1])
                    nc.scalar.dma_start(
                        out=out[pn * 128:(pn + 1) * 128, hh * 512:(hh + 1) * 512],
                        in_=o_sb[:, hh * 512:(hh + 1) * 512])

            kn_prev = [None]

            def emit_blocks(u, tiles):
                qT_u, kT_u, v_u, vp_u = tiles
                # ---- block loop. State S~ is updated at PAIR granularity:
                # the kv matmuls of blocks (2m, 2m+1) accumulate into one
                # PSUM tile per chunk (halving the DVE state-adds, which
                # otherwise rate-limit the PE through the PSUM rotation).
                # Odd blocks add the missing q_n @ kv_{n-1} term directly:
                #   y2_miss = vp_{n-1}^T (k_{n-1} q_n^T)   [no mask needed:
                # vscale x escale supply exactly (1-d) d^(128+t-s)].
                for j in range(JB):
                    n = u * JB + j
                    jsl = slice(j * 128, (j + 1) * 128)
                    odd = (n % 2 == 1)

                    # intra-block attention logits a[s,t], masked
                    a_ps = psa.tile([128, 128], F32, name="a_ps")
                    for c in range(KC):
                        nc.tensor.matmul(
                            a_ps, kT_u[:, c, jsl], qT_u[:, c, jsl],
                            start=(c == 0), stop=(c == KC - 1))
                    a_sb = ap_pool.tile([128, 128], BF, name="a_sb")
                    nc.vector.tensor_mul(a_sb, a_ps, mk(n))

                    # cross-pair logits for the odd-block correction
                    if odd:
                        pjsl = slice((j - 1) * 128, j * 128)
                        ax_ps = psB.tile([128, 128], F32, name="ax_ps", tag="psB")
                        for c in range(KC):
                            nc.tensor.matmul(
                                ax_ps, kT_u[:, c, pjsl], qT_u[:, c, jsl],
                                start=(c == 0), stop=(c == KC - 1))
                        ax_sb = ap_pool.tile([128, 128], BF, name="ax_sb")
                        nc.vector.tensor_copy(ax_sb, ax_ps)

                    # k natural [s, dk] via PE transposes of kT (state path;
                    # not needed for the final pair of blocks)
                    if n < N - 2:
                        kn = kn_pool.tile([128, Dk], BF, name="kn")
                        for h in range(KC // 4):
                            tp = psB.tile([128, 1024], BF, name="tp", tag="psB")[:, :512]
                            for q4 in range(4):
                                c = h * 4 + q4
                                nc.tensor.transpose(
                                    tp[:, q4 * 128:(q4 + 1) * 128],
                                    kT_u[:, c, jsl], id_sb)
                            nc.vector.tensor_copy(kn[:, h * 512:(h + 1) * 512], tp)
                    else:
                        kn = None

                    # out-projection of the PREVIOUS block (pipelined)
                    if prev[0] is not None:
                        emit_out(prev[0], prev[1])
                        prev[0] = None

                    # yT[dv, t] = v^T a~ + S~^T-contract + pair correction
                    y_ps = psY.tile([128, 4 * 128], F32, name="y_ps", tag="psY")
                    for dc in range(DC):
                        osl = slice(dc * 128, (dc + 1) * 128)
                        dvsl = slice(dc * 128, (dc + 1) * 128)
                        nc.tensor.matmul(
                            y_ps[:, osl], v_u[:, j, dvsl], a_sb,
                            start=True, stop=(n == 0))
                        if n >= 2:
                            for c in range(KC):
                                nc.tensor.matmul(
                                    y_ps[:, osl], S_c[c][:, dvsl], qT_u[:, c, jsl],
                                    start=False, stop=(not odd and c == KC - 1))
                        if odd:
                            nc.tensor.matmul(
                                y_ps[:, osl], vp_u[:, j - 1, dvsl], ax_sb,
                                start=False, stop=True)

                    # paired state update, emitted after the y matmuls (the
                    # DVE adds have a WAR dependency on this block's y2
                    # reads of S~, so emitting them earlier would stall the
                    # in-order PE queue through the PSUM rotation)
                    if odd and n <= N - 3:
                        for c in range(KC):
                            kv_ps = psB.tile([128, 512], F32, name="kv_ps",
                                             tag="psB")[:, :Dvh]
                            nc.tensor.matmul(
                                kv_ps, kn_prev[0][:, c * 128:(c + 1) * 128],
                                vp_u[:, j - 1, :], start=True, stop=False)
                            nc.tensor.matmul(
                                kv_ps, kn[:, c * 128:(c + 1) * 128],
                                vp_u[:, j, :], start=False, stop=True)
                            if n == 1:
                                nc.vector.tensor_copy(S_c[c], kv_ps)
                            else:
                                nc.vector.tensor_add(S_c[c], S_c[c], kv_ps)

                    yT_sb = yt_pool.tile([128, 4 * 128], BF, name="yT_sb")
                    nc.scalar.copy(yT_sb, y_ps)
                    prev[0], prev[1] = yT_sb, n
                    kn_prev[0] = kn

            # Software pipeline, depth 2: phase A (and its AllGather) for
            # u+2 is issued before the block loop of u. The v projection of
            # u is emitted just before its block loop.
            pend = {0: emit_phase_a_full(0, xt8_0c)}
            if NU > 1:
                pend[1] = emit_phase_a_full(1, xt8_1)
            for u in range(NU):
                v_tiles = emit_v(u, pend.pop(u))
                if u + 2 < NU:
                    pend[u + 2] = emit_phase_a(u + 2)
                emit_blocks(u, v_tiles)
            emit_out(prev[0], prev[1])
    return nc


def make_host_constants(T=4096, dtype_np=np.float32):
    """Host-precomputed decay constants (see module docstring)."""
    N = T // TBLK
    d = np.float64(DECAY)
    D128 = d ** TBLK
    s = np.arange(TBLK, dtype=np.float64)
    t = np.arange(TBLK, dtype=np.float64)
    nn = np.arange(N, dtype=np.float64)

    # masks[s, n, t] = 1[s<=t] * d^(-s-1) * D128^-(n-1)
    tri = (s[:, None] <= t[None, :]).astype(np.float64)  # [s, t]
    m = tri[:, None, :] * (d ** (-s - 1.0))[:, None, None] \
        * (D128 ** (-(nn - 1.0)))[None, :, None]
    masks = m.astype(BF16)

    # vscale[t, n] = (1-d) d^(127-t) D128^-n
    vsc = ((1.0 - d) * d ** (127.0 - t))[:, None] * (D128 ** (-nn))[None, :]
    vscale = vsc.astype(np.float32)

    # escale[t, n] = d^(t+1) D128^(n-1)
    esc = (d ** (t + 1.0))[:, None] * (D128 ** (nn - 1.0))[None, :]
    escale = esc.astype(np.float32)

    ident = np.eye(128, dtype=BF16)
    return masks, vscale, escale, ident


_NC_CACHE = {}


def _get_nc(T, E, Dk, Dvh):
    key = (T, E, Dk, Dvh)
    if key not in _NC_CACHE:
        nc = build_nc(T=T, E=E, Dk=Dk, Dvh=Dvh)
        nc.finalize()
        _NC_CACHE[key] = nc
    return _NC_CACHE[key]


def kernel(x, Wv, Wk, bk, Wq, bq, Wo):
    y, _ = run(x, Wv, Wk, bk, Wq, bq, Wo)
    return y


def _install_ntff_hook():
    """The agent image's antenv lacks axon_hooks; recreate it from
    trn_boot's ctypes NTFF driver so trace=True produces profiles."""
    try:
        from antenv.axon_hooks import get_axon_ntff_profile_hook  # noqa: F401
        return
    except ImportError:
        pass
    try:
        import types
        import antenv
        from trn_agent_boot.trn_boot import _ntff_profile_via_ctypes
        hook = _ntff_profile_via_ctypes("/opt/axon/libaxon_pjrt.so")
        mod = types.ModuleType("antenv.axon_hooks")
        _h = {"hook": hook}
        mod.get_axon_ntff_profile_hook = lambda: _h["hook"]
        mod.set_axon_ntff_profile_hook = lambda h: _h.update(hook=h)
        sys.modules["antenv.axon_hooks"] = mod
        antenv.axon_hooks = mod
    except Exception as e:  # profiling is best-effort
        print(f"ntff hook install failed: {e}")


def _arrange_xt(xb, ST=512, dtype=BF16):
    """x[b] [T, E] -> xT pre-tiled [NU, 128, EC, ST], contiguous."""
    T, E = xb.shape
    xT = np.ascontiguousarray(xb.T).astype(dtype)         # [E, T]
    EC, NU = E // 128, T // ST
    return np.ascontiguousarray(
        xT.reshape(EC, 128, NU, ST).transpose(2, 1, 0, 3))


def _arrange_w(w):
    """[E-or-Dv, D] -> [128, chunks, D] with row = chunk*128 + p."""
    R, D = w.shape
    C = R // 128
    return np.ascontiguousarray(w.reshape(C, 128, D).transpose(1, 0, 2))


def _arrange_b(b):
    b = np.asarray(b, np.float32).reshape(-1)
    C = b.shape[0] // 128
    return np.ascontiguousarray(b.reshape(C, 128).T)


def run(x, Wv, Wk, bk, Wq, bq, Wo, trace=False):
    x = np.asarray(x)
    B, T, E = x.shape
    Dk = np.asarray(Wk).shape[1]
    Dv = np.asarray(Wv).shape[1]
    Dvh = Dv // 2
    assert B == 4, "sharding is hardcoded for B=4 x 2 Dv-halves"

    nc = _get_nc(T, E, Dk, Dvh)
    masks, vscale, escale, ident = make_host_constants(T=T)

    # q/k weights scaled into e4m3's normal range; the kernel's sigmoid
    # activation divides the scale back out
    wq_f8 = np.asarray(np.asarray(Wq, np.float32) * WSCALE, E4M3)
    wk_f8 = np.asarray(np.asarray(Wk, np.float32) * WSCALE, E4M3)
    bq32 = np.asarray(bq, np.float32).reshape(Dk, 1)
    bk32 = np.asarray(bk, np.float32).reshape(Dk, 1)
    Dkh = Dk // 2

    xt_b = [_arrange_xt(x[b]) for b in range(B)]
    xt8_b = [_arrange_xt(x[b], dtype=E4M3) for b in range(B)]
    wqf_a = np.stack([_arrange_w(wq_f8[:, :Dkh]), _arrange_w(wq_f8[:, Dkh:])])
    wkf_a = np.stack([_arrange_w(wk_f8[:, :Dkh]), _arrange_w(wk_f8[:, Dkh:])])
    bqf_a = _arrange_b(bq32)
    bkf_a = _arrange_b(bk32)
    in_maps = []
    for c in range(8):
        b, h = divmod(c, 2)
        dvs = slice(h * Dvh, (h + 1) * Dvh)
        # this core computes the q/k dk-half matching its pair rank
        dks = slice(h * Dkh, (h + 1) * Dkh)
        in_maps.append({
            "xt": xt_b[b],
            "xt8": xt8_b[b],
            "wqf": wqf_a,
            "wkf": wkf_a,
            "bqf": bqf_a,
            "bkf": bkf_a,
            "wq": _arrange_w(wq_f8[:, dks]),
            "wk": _arrange_w(wk_f8[:, dks]),
            "wv": _arrange_w(np.asarray(Wv[:, dvs], BF16)),
            "wo": _arrange_w(np.asarray(Wo[dvs], BF16)),
            "bq": _arrange_b(bq32[dks]),
            "bk": _arrange_b(bk32[dks]),
            "masks": masks,
            "vscale": vscale,
            "escale": escale,
            "ident": ident,
        })

    if trace:
        _install_ntff_hook()
    res = run_bass_kernel_spmd(nc, in_maps, core_ids=list(range(8)), trace=trace)
    y = np.zeros((B, T, E), np.float32)
    for c in range(8):
        b = c // 2
        y[b] += np.asarray(res.results[c]["out"], np.float32)
    return y, res


# revision 22
# speedup vs baseline: 1.0028x; 1.0028x over previous
"""Trainium2 Bass kernel for nn_Decay2DBlk (block-decay linear attention).

Full-input contract: kernel(**inputs) takes the unsharded inputs from
setup_inputs() and returns the full [B, T, E] output.

Sharding: 8 cores = 4 batch elements x 2 Dv-halves. Each core computes a
partial output y_b_h = (attn(x_b) restricted to its Dv half) @ Wo[half];
the host sums the two partials per batch element (the "all-reduce after
w_out" done host-side since outputs are gathered anyway).

Math (per core): with d=0.99, D=d^128, block index n, in-block offsets
s,t (keys/queries):
  out[t] = sum_{s<=t} d^(t_g - s_g) * q_t k_s * v_s  @ Wo      (t_g global)
All decay factors are folded into host-precomputed constants so the device
only does matmuls + one elementwise mask multiply + a running-sum state:
  - masks[n][s,t]  = 1[s<=t] d^(-s-1) D^-(n-1)        (intra-block, bf16)
  - vscale[n][t]   = (1-d) d^(127-t) D^-n             (v' for state update)
  - escale[n][t]   = d^(t+1) D^(n-1)                  (final ACT evac scale)
  - state S~ = sum_m k_m^T v'_m  (pure running sum, bf16 in SBUF)
The geometric growth of D^-n keeps every intermediate within fp32/bf16
range (max ~1e19) and makes bf16 rounding of the running sum benign
(validated: rel err ~4e-3 vs fp32 reference).

Perf structure (v3):
  - supertiles 0/1 compute BOTH q/k dk-halves locally (skips the first
    AllGathers, which otherwise head-of-line block the PE for ~28us);
    supertiles 2..7 split q/k across the pair and exchange via AllGather
    overlapped with compute (kT gather on the sync ring, qT on gpsimd).
  - state S~ updated at block-PAIR granularity: the kv matmuls of blocks
    (2m, 2m+1) accumulate in PSUM, halving the DVE tensor-adds that
    otherwise rate-limit the PE through the PSUM-bank rotation. Odd
    blocks add the missing q_n kv_{n-1} term as vp_{n-1}^T (k_{n-1} q_n^T)
    (mask-free: the vscale/escale constants supply (1-d) d^(128+t-s)).
  - inputs spread across the three DMA-capable rings (sync/scalar/gpsimd,
    ~76 GB/s each) in need-order; out stores are bf16 (host upcasts).
  - out-projection software-pipelined one block behind the y matmuls.
"""

import os
import sys

for _p in (
    "/root/.axon_site",
    "/root/.axon_site/_ro/trn_rl_repo",
    "/root/.axon_site/_ro/pypackages",
    "/opt/trn_rl_repo",
):
    if os.path.isdir(_p) and _p not in sys.path:
        sys.path.append(_p)

import numpy as np
import ml_dtypes
from contextlib import ExitStack

import concourse.bacc as bacc
import concourse.tile as tile
from concourse import mybir
from concourse.bass_utils import run_bass_kernel_spmd

DECAY = 0.99
TBLK = 128
BF16 = ml_dtypes.bfloat16
E4M3 = ml_dtypes.float8_e4m3
BF = mybir.dt.bfloat16
F32 = mybir.dt.float32
F8 = mybir.dt.float8e4
# fp8 scale on Wq/Wk so the N(0, 0.02^2) weights land in e4m3's normal
# range (std 1.28); undone by the sigmoid activation's scale=1/WSCALE
WSCALE = 64.0


def build_nc(T=4096, E=1024, Dk=1024, Dvh=512, ST=512, pair_groups=None):
    """Build the per-core Bass program. Same program runs on all 8 cores
    (SPMD); only the input data differs.

    For u>=2, q/k phase-A work is split across the two cores of a pair:
    each core computes sigmoid(x @ Wq_half + b_half) for the dk-half whose
    weights it was GIVEN as input (wq input is [E, Dk/2]), then the halves
    are exchanged with a pairwise AllGather through DRAM bounce buffers.
    Core 2b (group rank 0) always carries the low half, so the gathered
    layout is identical on both cores and the program stays SPMD-symmetric.
    Supertiles 0 and 1 instead compute the full Dk locally from wqf/wkf
    (same global chunk order as the gathered layout) so no collective sits
    on the critical path at startup."""
    N = T // TBLK       # number of 128-token blocks
    NU = T // ST        # number of super-tiles
    JB = ST // TBLK     # blocks per super-tile
    EC = E // 128       # E chunks (contraction)
    KC = Dk // 128      # Dk chunks
    HC = KC // 2        # dk chunks computed locally (half)
    DC = Dvh // 128     # Dv-half chunks
    Dkh = Dk // 2
    if pair_groups is None:
        pair_groups = [[0, 1], [2, 3], [4, 5], [6, 7]]

    nc = bacc.Bacc(num_devices=8)
    # all inputs host-pre-arranged to [128-partition, ...contiguous] layout so
    # every DMA is 128 descriptors of 4-8KB (max descriptor efficiency)
    xt = nc.dram_tensor("xt", [NU, 128, EC, ST], BF, kind="ExternalInput")
    xt8 = nc.dram_tensor("xt8", [NU, 128, EC, ST], F8, kind="ExternalInput")
    # full q/k weights (global chunk order, identical on both pair cores)
    wqf = nc.dram_tensor("wqf", [2, 128, EC, Dkh], F8, kind="ExternalInput")
    wkf = nc.dram_tensor("wkf", [2, 128, EC, Dkh], F8, kind="ExternalInput")
    bqf = nc.dram_tensor("bqf", [128, KC], F32, kind="ExternalInput")
    bkf = nc.dram_tensor("bkf", [128, KC], F32, kind="ExternalInput")
    # per-core halves (this core's share for the u>=2 split phase A)
    wq = nc.dram_tensor("wq", [128, EC, Dkh], F8, kind="ExternalInput")
    wk = nc.dram_tensor("wk", [128, EC, Dkh], F8, kind="ExternalInput")
    wv = nc.dram_tensor("wv", [128, EC, Dvh], BF, kind="ExternalInput")
    wo = nc.dram_tensor("wo", [128, DC, E], BF, kind="ExternalInput")
    bq = nc.dram_tensor("bq", [128, HC], F32, kind="ExternalInput")
    bk = nc.dram_tensor("bk", [128, HC], F32, kind="ExternalInput")
    masks = nc.dram_tensor("masks", [128, N, 128], BF, kind="ExternalInput")
    vscale = nc.dram_tensor("vscale", [128, N], F32, kind="ExternalInput")
    escale = nc.dram_tensor("escale", [128, N], F32, kind="ExternalInput")
    ident = nc.dram_tensor("ident", [128, 128], BF, kind="ExternalInput")
    out = nc.dram_tensor("out", [T, E], BF, kind="ExternalOutput")

    SIG = mybir.ActivationFunctionType.Sigmoid
    CPY = mybir.ActivationFunctionType.Copy

    with tile.TileContext(nc) as tc:
        with ExitStack() as ctx:
            consts = ctx.enter_context(tc.tile_pool(name="consts", bufs=1))
            qkh_pool = ctx.enter_context(tc.tile_pool(name="qkh", bufs=2))
            dram = ctx.enter_context(tc.tile_pool(name="dram", bufs=2, space="DRAM"))
            xt_pool = ctx.enter_context(tc.tile_pool(name="xt", bufs=3))
            qk_pool = ctx.enter_context(tc.tile_pool(name="qk", bufs=3))
            v_pool = ctx.enter_context(tc.tile_pool(name="v", bufs=3))
            kn_pool = ctx.enter_context(tc.tile_pool(name="kn", bufs=3))
            ap_pool = ctx.enter_context(tc.tile_pool(name="apool", bufs=3))
            yt_pool = ctx.enter_context(tc.tile_pool(name="yt", bufs=3))
            o_pool = ctx.enter_context(tc.tile_pool(name="opool", bufs=2))
            st_pool = ctx.enter_context(tc.tile_pool(name="state", bufs=1))
            psA = ctx.enter_context(tc.tile_pool(name="psA", bufs=2, space="PSUM"))
            psB = ctx.enter_context(tc.tile_pool(name="psB", bufs=3, space="PSUM"))
            psY = ctx.enter_context(tc.tile_pool(name="psY", bufs=2, space="PSUM"))
            psa = ctx.enter_context(tc.tile_pool(name="psa", bufs=1, space="PSUM"))

            # ---- constants into SBUF ----
            # One trigger-ring queue sustains only ~76 GB/s, so the inputs
            # are spread across the three DMA-capable rings in need-order:
            #   sync ring:   xt8[0], wqf halves, xt8[1], wq, wk,
            #                then per-supertile xt8[u] + gathered-kT loads
            #   scalar ring: biases, wkf halves, early masks, scales,
            #                then per-supertile xt[u] (bf16) + out stores
            #   gpsimd ring: ident, xt[0], wv, wo, then AllGather bounces
            # q/k full weights split by OUTPUT columns: the first dk-half
            # of q-chunks is runnable after only (xt8[0] + 0.5 MB) lands
            bqf_sb = consts.tile([128, KC], F32)
            nc.scalar.dma_start(out=bqf_sb, in_=bqf[:, :])
            bkf_sb = consts.tile([128, KC], F32)
            nc.scalar.dma_start(out=bkf_sb, in_=bkf[:, :])
            bq_sb = consts.tile([128, HC], F32)
            nc.scalar.dma_start(out=bq_sb, in_=bq[:, :])
            bk_sb = consts.tile([128, HC], F32)
            nc.scalar.dma_start(out=bk_sb, in_=bk[:, :])
            xt8_0 = xt_pool.tile([128, EC, ST], F8, name="xt8_u", tag="xt8")
            nc.sync.dma_start(out=xt8_0, in_=xt8[0])
            wqf_sb = [consts.tile([128, EC, Dkh], F8, name=f"wqf{h}") for h in range(2)]
            wkf_sb = [consts.tile([128, EC, Dkh], F8, name=f"wkf{h}") for h in range(2)]
            for h in range(2):
                nc.sync.dma_start(out=wqf_sb[h], in_=wqf[h])
                nc.scalar.dma_start(out=wkf_sb[h], in_=wkf[h])
            xt8_1 = xt_pool.tile([128, EC, ST], F8, name="xt8_u", tag="xt8")
            nc.sync.dma_start(out=xt8_1, in_=xt8[1])
            # u>=2 half weights behind xt8[1] on the sync ring
            wq_sb = consts.tile([128, EC, Dkh], F8)
            nc.sync.dma_start(out=wq_sb, in_=wq[:, :, :])
            wk_sb = consts.tile([128, EC, Dkh], F8)
            nc.sync.dma_start(out=wk_sb, in_=wk[:, :, :])
            # masks split so the first two supertiles' worth arrives early;
            # the rest is triggered later (inside phase_a(3)) so it never
            # delays the per-supertile xt streams
            NM0 = 2 * JB
            mk0_sb = consts.tile([128, NM0, 128], BF)
            nc.scalar.dma_start(out=mk0_sb, in_=masks[:, :NM0, :])
            vs_sb = consts.tile([128, N], F32)
            nc.scalar.dma_start(out=vs_sb, in_=vscale[:, :])
            es_sb = consts.tile([128, N], F32)
            nc.scalar.dma_start(out=es_sb, in_=escale[:, :])
            mk1_sb = consts.tile([128, N - NM0, 128], BF)
            id_sb = consts.tile([128, 128], BF)
            nc.gpsimd.dma_start(out=id_sb, in_=ident[:, :])
            xt_0 = xt_pool.tile([128, EC, ST], BF, name="xt_u", tag="xt")
            nc.gpsimd.dma_start(out=xt_0, in_=xt[0])
            wv_sb = consts.tile([128, EC, Dvh], BF)
            nc.gpsimd.dma_start(out=wv_sb, in_=wv[:, :, :])
            wo_sb = consts.tile([128, DC, E], BF)
            nc.gpsimd.dma_start(out=wo_sb, in_=wo[:, :, :])

            def mk(n):
                return mk0_sb[:, n, :] if n < NM0 else mk1_sb[:, n - NM0, :]

            # persistent scaled-sum state S~ [dk, dv], one tile per dk-chunk
            S_c = [st_pool.tile([128, Dvh], BF, name=f"S{c}", tag=f"S{c}")
                   for c in range(KC)]

            def emit_phase_a_full(u, xt8_u):
                # ---- supertiles 0/1: full q/k computed locally, no
                # AllGather on the startup critical path.
                if u == 0:
                    xt_u = xt_0
                else:
                    xt_u = xt_pool.tile([128, EC, ST], BF, name="xt_u", tag="xt")
                    nc.scalar.dma_start(out=xt_u, in_=xt[u])
                qT_u = qk_pool.tile([128, KC, ST], BF, name="qT_u", tag="qT")
                kT_u = qk_pool.tile([128, KC, ST], BF, name="kT_u", tag="kT")
                for wsb, bsb, dst in ((wqf_sb, bqf_sb, qT_u), (wkf_sb, bkf_sb, kT_u)):
                    for c in range(KC):
                        csl = slice((c % (KC // 2)) * 128, (c % (KC // 2) + 1) * 128)
                        ps = psA.tile([128, ST], F32, name="psq", tag="psA")
                        for e in range(0, EC, 2):
                            nc.tensor.matmul(
                                ps, wsb[c // (KC // 2)][:, e:e + 2, csl],
                                xt8_u[:, e:e + 2, :],
                                start=(e == 0), stop=(e == EC - 2),
                                perf_mode=mybir.MatmulPerfMode.DoubleRow)
                        nc.scalar.activation(dst[:, c, :], ps, SIG,
                                             bias=bsb[:, c:c + 1], scale=1.0 / WSCALE)
                return qT_u, kT_u, xt_u

            def emit_v(u, tiles):
                # ---- v projection for supertile u (bf16: fp8 x would put
                # ~3.7% error straight onto the value path). Emitted right
                # before its block loop; the xt[u] bf16 stream was triggered
                # two supertiles earlier.
                qT_u, kT_u, xt_u = tiles
                v_u = v_pool.tile([128, JB, Dvh], BF, name="v_u", tag="v")
                vp_u = v_pool.tile([128, JB, Dvh], BF, name="vp_u", tag="vp")
                for j in range(JB):
                    n = u * JB + j
                    ps = psA.tile([128, ST], F32, name="psv", tag="psA")[:, :Dvh]
                    for e in range(EC):
                        nc.tensor.matmul(
                            ps, xt_u[:, e, j * 128:(j + 1) * 128], wv_sb[:, e, :],
                            start=(e == 0), stop=(e == EC - 1))
                    nc.scalar.activation(v_u[:, j, :], ps, CPY, scale=1.0 - DECAY)
                    nc.scalar.activation(vp_u[:, j, :], ps, CPY, scale=vs_sb[:, n:n + 1])
                return qT_u, kT_u, v_u, vp_u

            def emit_phase_a(u):
                # xt streams for supertile u ride the sync (fp8) and scalar
                # (bf16) rings, triggered here = two supertiles early.
                xt8_u = xt_pool.tile([128, EC, ST], F8, name="xt8_u", tag="xt8")
                nc.sync.dma_start(out=xt8_u, in_=xt8[u])
                xt_u = xt_pool.tile([128, EC, ST], BF, name="xt_u", tag="xt")
                nc.scalar.dma_start(out=xt_u, in_=xt[u])
                if u == 3:
                    nc.scalar.dma_start(out=mk1_sb, in_=masks[:, NM0:, :])

                # ---- phase A: this core computes its dk-half of qT, kT ----
                # fp8 DoubleRow: two 128-deep contraction subtiles per pass
                qTh_u = qkh_pool.tile([128, HC, ST], BF, name="qTh_u", tag="qTh")
                kTh_u = qkh_pool.tile([128, HC, ST], BF, name="kTh_u", tag="kTh")
                for c in range(HC):
                    csl = slice(c * 128, (c + 1) * 128)
                    ps = psA.tile([128, ST], F32, name="psq", tag="psA")
                    for e in range(0, EC, 2):
                        nc.tensor.matmul(
                            ps, wq_sb[:, e:e + 2, csl], xt8_u[:, e:e + 2, :],
                            start=(e == 0), stop=(e == EC - 2),
                            perf_mode=mybir.MatmulPerfMode.DoubleRow)
                    nc.scalar.activation(qTh_u[:, c, :], ps, SIG,
                                         bias=bq_sb[:, c:c + 1], scale=1.0 / WSCALE)
                for c in range(HC):
                    csl = slice(c * 128, (c + 1) * 128)
                    ps = psA.tile([128, ST], F32, name="psk", tag="psA")
                    for e in range(0, EC, 2):
                        nc.tensor.matmul(
                            ps, wk_sb[:, e:e + 2, csl], xt8_u[:, e:e + 2, :],
                            start=(e == 0), stop=(e == EC - 2),
                            perf_mode=mybir.MatmulPerfMode.DoubleRow)
                    nc.scalar.activation(kTh_u[:, c, :], ps, SIG,
                                         bias=bk_sb[:, c:c + 1], scale=1.0 / WSCALE)

                # ---- exchange halves with the pair partner (AllGather) ----
                # bounce-out + qT gather ride gpsimd; the kT gather rides the
                # sync ring (its AllGather-completion wait resolves ~a whole
                # supertile before anything queued behind it is needed)
                bin_u = dram.tile([2, 128, HC, ST], BF, name="bin_u", tag="bin")
                nc.gpsimd.dma_start(out=bin_u[0], in_=qTh_u)
                nc.gpsimd.dma_start(out=bin_u[1], in_=kTh_u)
                bout_u = dram.tile([2, 2, 128, HC, ST], BF, name="bout_u", tag="bout")
                nc.gpsimd.collective_compute(
                    "AllGather", mybir.AluOpType.bypass,
                    replica_groups=pair_groups,
                    ins=[bin_u.opt()], outs=[bout_u.opt()])
                qT_u = qk_pool.tile([128, KC, ST], BF, name="qT_u", tag="qT")
                kT_u = qk_pool.tile([128, KC, ST], BF, name="kT_u", tag="kT")
                for g in range(2):
                    nc.sync.dma_start(
                        out=kT_u[:, g * HC:(g + 1) * HC, :], in_=bout_u[g, 1])
                    nc.gpsimd.dma_start(
                        out=qT_u[:, g * HC:(g + 1) * HC, :], in_=bout_u[g, 0])
                return qT_u, kT_u, xt_u

            # out-projection is software-pipelined one block behind: the
            # yT evac (scalar) gets the next block a/kn matmuls to finish
            # under before the out matmuls need it as stationary.
            prev = [None, None]   # (yT_sb, n)

            def emit_out(prev_yT, pn):
                # out[t, e] = yT^T @ Wo, evacuated with escale[n][t]
                o_sb = o_pool.tile([128, E], BF, name="o_sb")
                for hh in range(E // 512):
                    o_ps = psY.tile([128, 4 * 128], F32, name="o_ps", tag="psY")
                    for dc in range(DC):
                        nc.tensor.matmul(
                            o_ps, prev_yT[:, dc * 128:(dc + 1) * 128],
                            wo_sb[:, dc, hh * 512:(hh + 1) * 512],
                            start=(dc == 0), stop=(dc == DC - 1))
                    nc.scalar.activation(
                        o_sb[:, hh * 512:(hh + 1) * 512], o_ps, CPY,
                        scale=es_sb[:, pn:pn + 1])
                    nc.scalar.dma_start(
                        out=out[pn * 128:(pn + 1) * 128, hh * 512:(hh + 1) * 512],
                        in_=o_sb[:, hh * 512:(hh + 1) * 512])

            kn_prev = [None]

            def emit_blocks(u, tiles):
                qT_u, kT_u, v_u, vp_u = tiles
                # ---- block loop. State S~ is updated at PAIR granularity:
                # the kv matmuls of blocks (2m, 2m+1) accumulate into one
                # PSUM tile per chunk (halving the DVE state-adds, which
                # otherwise rate-limit the PE through the PSUM rotation).
                # Odd blocks add the missing q_n @ kv_{n-1} term directly:
                #   y2_miss = vp_{n-1}^T (k_{n-1} q_n^T)   [no mask needed:
                # vscale x escale supply exactly (1-d) d^(128+t-s)].
                for j in range(JB):
                    n = u * JB + j
                    jsl = slice(j * 128, (j + 1) * 128)
                    odd = (n % 2 == 1)

                    # intra-block attention logits a[s,t], masked
                    a_ps = psa.tile([128, 128], F32, name="a_ps")
                    for c in range(KC):
                        nc.tensor.matmul(
                            a_ps, kT_u[:, c, jsl], qT_u[:, c, jsl],
                            start=(c == 0), stop=(c == KC - 1))
                    a_sb = ap_pool.tile([128, 128], BF, name="a_sb")
                    nc.vector.tensor_mul(a_sb, a_ps, mk(n))

                    # cross-pair logits for the odd-block correction
                    if odd:
                        pjsl = slice((j - 1) * 128, j * 128)
                        ax_ps = psB.tile([128, 128], F32, name="ax_ps", tag="psB")
                        for c in range(KC):
                            nc.tensor.matmul(
                                ax_ps, kT_u[:, c, pjsl], qT_u[:, c, jsl],
                                start=(c == 0), stop=(c == KC - 1))
                        ax_sb = ap_pool.tile([128, 128], BF, name="ax_sb")
                        nc.vector.tensor_copy(ax_sb, ax_ps)

                    # k natural [s, dk] via PE transposes of kT (state path;
                    # not needed for the final pair of blocks)
                    if n < N - 2:
                        kn = kn_pool.tile([128, Dk], BF, name="kn")
                        for h in range(KC // 4):
                            tp = psB.tile([128, 1024], BF, name="tp", tag="psB")[:, :512]
                            for q4 in range(4):
                                c = h * 4 + q4
                                nc.tensor.transpose(
                                    tp[:, q4 * 128:(q4 + 1) * 128],
                                    kT_u[:, c, jsl], id_sb)
                            nc.vector.tensor_copy(kn[:, h * 512:(h + 1) * 512], tp)
                    else:
                        kn = None

                    # out-projection of the PREVIOUS block (pipelined)
                    if prev[0] is not None:
                        emit_out(prev[0], prev[1])
                        prev[0] = None

                    # yT[dv, t] = v^T a~ + S~^T-contract + pair correction
                    y_ps = psY.tile([128, 4 * 128], F32, name="y_ps", tag="psY")
                    for dc in range(DC):
                        osl = slice(dc * 128, (dc + 1) * 128)
                        dvsl = slice(dc * 128, (dc + 1) * 128)
                        nc.tensor.matmul(
                            y_ps[:, osl], v_u[:, j, dvsl], a_sb,
                            start=True, stop=(n == 0))
                        if n >= 2:
                            for c in range(KC):
                                nc.tensor.matmul(
                                    y_ps[:, osl], S_c[c][:, dvsl], qT_u[:, c, jsl],
                                    start=False, stop=(not odd and c == KC - 1))
                        if odd:
                            nc.tensor.matmul(
                                y_ps[:, osl], vp_u[:, j - 1, dvsl], ax_sb,
                                start=False, stop=True)

                    # paired state update, emitted after the y matmuls (the
                    # DVE adds have a WAR dependency on this block's y2
                    # reads of S~, so emitting them earlier would stall the
                    # in-order PE queue through the PSUM rotation)
                    if odd and n <= N - 3:
                        for c in range(KC):
                            kv_ps = psB.tile([128, 512], F32, name="kv_ps",
                                             tag="psB")[:, :Dvh]
                            nc.tensor.matmul(
                                kv_ps, kn_prev[0][:, c * 128:(c + 1) * 128],
                                vp_u[:, j - 1, :], start=True, stop=False)
                            nc.tensor.matmul(
                                kv_ps, kn[:, c * 128:(c + 1) * 128],
                                vp_u[:, j, :], start=False, stop=True)
                            if n == 1:
                                nc.vector.tensor_copy(S_c[c], kv_ps)
                            else:
                                nc.vector.tensor_add(S_c[c], S_c[c], kv_ps)

                    yT_sb = yt_pool.tile([128, 4 * 128], BF, name="yT_sb")
                    nc.scalar.copy(yT_sb, y_ps)
                    prev[0], prev[1] = yT_sb, n
                    kn_prev[0] = kn

            # Software pipeline, depth 2: phase A (and its AllGather) for
            # u+2 is issued before the block loop of u. The v projection of
            # u is emitted just before its block loop.
            pend = {0: emit_phase_a_full(0, xt8_0)}
            if NU > 1:
                pend[1] = emit_phase_a_full(1, xt8_1)
            for u in range(NU):
                v_tiles = emit_v(u, pend.pop(u))
                if u + 2 < NU:
                    pend[u + 2] = emit_phase_a(u + 2)
                emit_blocks(u, v_tiles)
            emit_out(prev[0], prev[1])
    return nc


def make_host_constants(T=4096, dtype_np=np.float32):
    """Host-precomputed decay constants (see module docstring)."""
    N = T // TBLK
    d = np.float64(DECAY)
    D128 = d ** TBLK
    s = np.arange(TBLK, dtype=np.float64)
    t = np.arange(TBLK, dtype=np.float64)
    nn = np.arange(N, dtype=np.float64)

    # masks[s, n, t] = 1[s<=t] * d^(-s-1) * D128^-(n-1)
    tri = (s[:, None] <= t[None, :]).astype(np.float64)  # [s, t]
    m = tri[:, None, :] * (d ** (-s - 1.0))[:, None, None] \
        * (D128 ** (-(nn - 1.0)))[None, :, None]
    masks = m.astype(BF16)

    # vscale[t, n] = (1-d) d^(127-t) D128^-n
    vsc = ((1.0 - d) * d ** (127.0 - t))[:, None] * (D128 ** (-nn))[None, :]
    vscale = vsc.astype(np.float32)

    # escale[t, n] = d^(t+1) D128^(n-1)
    esc = (d ** (t + 1.0))[:, None] * (D128 ** (nn - 1.0))[None, :]
    escale = esc.astype(np.float32)

    ident = np.eye(128, dtype=BF16)
    return masks, vscale, escale, ident


_NC_CACHE = {}


def _get_nc(T, E, Dk, Dvh):
    key = (T, E, Dk, Dvh)
    if key not in _NC_CACHE:
        nc = build_nc(T=T, E=E, Dk=Dk, Dvh=Dvh)
        nc.finalize()
        _NC_CACHE[key] = nc
    return _NC_CACHE[key]


def kernel(x, Wv, Wk, bk, Wq, bq, Wo):
    y, _ = run(x, Wv, Wk, bk, Wq, bq, Wo)
    return y


def _install_ntff_hook():
    """The agent image's antenv lacks axon_hooks; recreate it from
    trn_boot's ctypes NTFF driver so trace=True produces profiles."""
    try:
        from antenv.axon_hooks import get_axon_ntff_profile_hook  # noqa: F401
        return
    except ImportError:
        pass
    try:
        import types
        import antenv
        from trn_agent_boot.trn_boot import _ntff_profile_via_ctypes
        hook = _ntff_profile_via_ctypes("/opt/axon/libaxon_pjrt.so")
        mod = types.ModuleType("antenv.axon_hooks")
        _h = {"hook": hook}
        mod.get_axon_ntff_profile_hook = lambda: _h["hook"]
        mod.set_axon_ntff_profile_hook = lambda h: _h.update(hook=h)
        sys.modules["antenv.axon_hooks"] = mod
        antenv.axon_hooks = mod
    except Exception as e:  # profiling is best-effort
        print(f"ntff hook install failed: {e}")


def _arrange_xt(xb, ST=512, dtype=BF16):
    """x[b] [T, E] -> xT pre-tiled [NU, 128, EC, ST], contiguous."""
    T, E = xb.shape
    xT = np.ascontiguousarray(xb.T).astype(dtype)         # [E, T]
    EC, NU = E // 128, T // ST
    return np.ascontiguousarray(
        xT.reshape(EC, 128, NU, ST).transpose(2, 1, 0, 3))


def _arrange_w(w):
    """[E-or-Dv, D] -> [128, chunks, D] with row = chunk*128 + p."""
    R, D = w.shape
    C = R // 128
    return np.ascontiguousarray(w.reshape(C, 128, D).transpose(1, 0, 2))


def _arrange_b(b):
    b = np.asarray(b, np.float32).reshape(-1)
    C = b.shape[0] // 128
    return np.ascontiguousarray(b.reshape(C, 128).T)


def run(x, Wv, Wk, bk, Wq, bq, Wo, trace=False):
    x = np.asarray(x)
    B, T, E = x.shape
    Dk = np.asarray(Wk).shape[1]
    Dv = np.asarray(Wv).shape[1]
    Dvh = Dv // 2
    assert B == 4, "sharding is hardcoded for B=4 x 2 Dv-halves"

    nc = _get_nc(T, E, Dk, Dvh)
    masks, vscale, escale, ident = make_host_constants(T=T)

    # q/k weights scaled into e4m3's normal range; the kernel's sigmoid
    # activation divides the scale back out
    wq_f8 = np.asarray(np.asarray(Wq, np.float32) * WSCALE, E4M3)
    wk_f8 = np.asarray(np.asarray(Wk, np.float32) * WSCALE, E4M3)
    bq32 = np.asarray(bq, np.float32).reshape(Dk, 1)
    bk32 = np.asarray(bk, np.float32).reshape(Dk, 1)
    Dkh = Dk // 2

    xt_b = [_arrange_xt(x[b]) for b in range(B)]
    xt8_b = [_arrange_xt(x[b], dtype=E4M3) for b in range(B)]
    wqf_a = np.stack([_arrange_w(wq_f8[:, :Dkh]), _arrange_w(wq_f8[:, Dkh:])])
    wkf_a = np.stack([_arrange_w(wk_f8[:, :Dkh]), _arrange_w(wk_f8[:, Dkh:])])
    bqf_a = _arrange_b(bq32)
    bkf_a = _arrange_b(bk32)
    in_maps = []
    for c in range(8):
        b, h = divmod(c, 2)
        dvs = slice(h * Dvh, (h + 1) * Dvh)
        # this core computes the q/k dk-half matching its pair rank
        dks = slice(h * Dkh, (h + 1) * Dkh)
        in_maps.append({
            "xt": xt_b[b],
            "xt8": xt8_b[b],
            "wqf": wqf_a,
            "wkf": wkf_a,
            "bqf": bqf_a,
            "bkf": bkf_a,
            "wq": _arrange_w(wq_f8[:, dks]),
            "wk": _arrange_w(wk_f8[:, dks]),
            "wv": _arrange_w(np.asarray(Wv[:, dvs], BF16)),
            "wo": _arrange_w(np.asarray(Wo[dvs], BF16)),
            "bq": _arrange_b(bq32[dks]),
            "bk": _arrange_b(bk32[dks]),
            "masks": masks,
            "vscale": vscale,
            "escale": escale,
            "ident": ident,
        })

    if trace:
        _install_ntff_hook()
    res = run_bass_kernel_spmd(nc, in_maps, core_ids=list(range(8)), trace=trace)
    y = np.zeros((B, T, E), np.float32)
    for c in range(8):
        b = c // 2
        y[b] += np.asarray(res.results[c]["out"], np.float32)
    return y, res
